# revision 11
# baseline (speedup 1.0000x reference)
"""2D orthonormal DCT-II over [32,64,224,224], data-parallel on 8 TRN2 cores.

Math per image X [224,224]:  Y = D @ X @ D.T  (D = 224-pt orthonormal DCT-II).
Uses the DCT cosine even/odd symmetry D[k, n-1-m] = (-1)^k D[k, m] to fold
both contractions to half size (4x fewer MACs than naive), and runs the
whole pipeline in bf16 (1 cyc/row on the PE at any stream length, half the
HBM traffic of fp32; rel err ~5e-3 vs the 2e-2 gate).

Host prep (layout + first butterfly, 0.2% of the FLOPs):
  u = X[0:112] + X[223:111:-1]   (rows folded)
  v = X[0:112] - X[223:111:-1]
  columns rearranged to [left 112 | right 112 reversed]; arrays stored
  partition-major ([h', img, ...]) so each DMA partition row is one
  contiguous multi-KB run per group.

Device, per 2-image block (all matmuls bf16, fp32 PSUM accum):
  stage 1 (data stationary): per image 8 MMs / 4 weight loads.
  The second butterfly (U = T_left + T_right, V = T_left - T_right) runs on
  the PE itself via PSUM accumulation -- the right-half matmul accumulates
  on top of the left-half one, with a sign-negated DCT stream for V.
  (DVE cannot add two PSUM operands, so folding on the PE is both legal
  and free: same stationary operand feeds the U and V streams.)
    U[w',(ke|ko)] = MM(u_l, DeT) + MM(u_r,  DeT)   (and v_*, DoT for ko)
    V[w',(ke|ko)] = MM(u_l, DeT) + MM(u_r, -DeT)
  evac U,V -> bf16 SBUF (Act / DVE unary copies)
  stage 2 (DCT stationary, 448-col streams over the 2-image block), run
  with a 2-block delay so the U/V evacuation latency is fully hidden:
    Ye[le,(img,k)] = De @ U      Yo[lo,(img,k)] = Do @ V     (2 MMs)
  evac Ye/Yo -> bf16 SBUF -> DRAM (partition-major); host de-interleaves.

PSUM: Up,Vp (1 bank each) + Ye,Yo (1 bank each), all double-buffered = 8
banks, so stage 1 of block i+1 never waits on the evacs of block i.
"""
import numpy as np
from ml_dtypes import bfloat16
import concourse.bacc as bacc
import concourse.mybir as mybir
import concourse.tile as tile
from concourse.bass_utils import run_bass_kernel_spmd

B, C, H, W = 32, 64, 224, 224
N_CORES = 8
IMGS = B * C // N_CORES  # 256 images per core
G = 8                    # images per DMA group
BLK = 2                  # images per PSUM block
NB = IMGS // BLK         # 128 blocks
H2 = 112                 # folded size

f32 = mybir.dt.float32
bf16 = mybir.dt.bfloat16

_cache = {}


def _dct2_matrix(n: int) -> np.ndarray:
    k = np.arange(n)[:, None].astype(np.float64)
    m = np.arange(n)[None, :].astype(np.float64)
    d = np.cos(np.pi * (2.0 * m + 1.0) * k / (2.0 * n))
    scale = np.full((n, 1), np.sqrt(2.0 / n))
    scale[0, 0] = np.sqrt(1.0 / n)
    return (scale * d).astype(np.float32)


def _build():
    nc = bacc.Bacc("TRN2", target_bir_lowering=False, debug=False)
    uv_d = nc.dram_tensor("uv", [H2, IMGS, 2, W], bf16, kind="ExternalInput").ap()
    de_d = nc.dram_tensor("de", [H2, 2, H2], bf16, kind="ExternalInput").ap()
    dn_d = nc.dram_tensor("dn", [H2, 2, H2], bf16, kind="ExternalInput").ap()
    yt_d = nc.dram_tensor("yt", [H2, IMGS, 2, W], bf16, kind="ExternalOutput").ap()

    with tile.TileContext(nc) as tc:
        with (
            tc.tile_pool(name="consts", bufs=1) as cpool,
            tc.tile_pool(name="xin", bufs=3) as xpool,
            tc.tile_pool(name="uv", bufs=3) as upool,
            tc.tile_pool(name="yout", bufs=2) as spool,
            tc.tile_pool(name="pst", bufs=2, space="PSUM") as pst,
            tc.tile_pool(name="psy", bufs=2, space="PSUM") as psy,
        ):
            # constants: DeT/DoT and negations [112, 2, 112]
            dct_s = cpool.tile([H2, 2, H2], bf16)
            dctn_s = cpool.tile([H2, 2, H2], bf16)
            nc.gpsimd.dma_start(dct_s, de_d)
            nc.gpsimd.dma_start(dctn_s, dn_d)
            det_s, dot_s = dct_s[:, 0, :], dct_s[:, 1, :]
            detn_s, dotn_s = dctn_s[:, 0, :], dctn_s[:, 1, :]

            # PE warmup: ~4us of junk matmuls to ramp the p-state before
            # the real work starts.
            junk_w = cpool.tile([H2, 128], bf16)
            junk_m = cpool.tile([H2, 448], bf16)
            nc.vector.memset(junk_w, 0)
            nc.vector.memset(junk_m, 0)
            for r in range(20):
                wp = psy.tile([128, BLK, W], f32, name=f"warm{r}", tag="ye")
                nc.tensor.matmul(wp[:, :, :], junk_w, junk_m,
                                 start=True, stop=True)

            def load_group(g):
                t = xpool.tile([H2, G, 2, W], bf16, name="tuv", tag="tuv")
                sl = slice(g * G, (g + 1) * G)
                nc.gpsimd.dma_start(t, uv_d[:, sl, :, :])
                return t

            BPG = G // BLK   # blocks per group
            DEPTH = 2        # s2 trails stage 1 by this many blocks
            states = []      # (US, VS, ys, b, g)
            pending = [load_group(0), load_group(1)]
            tuv = None
            ys = None

            def stage2_and_out(st):
                USp, VSp, ys_p, b_p, g_p = st
                ye = psy.tile([128, BLK, W], f32, name="ye", tag="ye")
                yo = psy.tile([128, BLK, W], f32, name="yo", tag="yo")
                nc.tensor.matmul(ye[0:H2, :, :], det_s,
                                 USp[:, :, :, :].transpose([0, 2, 1, 3]),
                                 start=True, stop=True)
                nc.tensor.matmul(yo[0:H2, :, :], dot_s,
                                 VSp[:, :, :, :].transpose([0, 2, 1, 3]),
                                 start=True, stop=True)
                j0 = b_p * BLK
                nc.scalar.copy(ys_p[:, j0:j0 + BLK, 0, :], ye[0:H2, :, :])
                nc.vector.tensor_copy(ys_p[:, j0:j0 + BLK, 1, :],
                                      yo[0:H2, :, :])
                if b_p % 2 == 1:
                    # flush a 4-image output chunk (keeps the out stream
                    # smooth and the final drain short)
                    ja = (b_p - 1) * BLK
                    slp = slice(g_p * G + ja, g_p * G + ja + 2 * BLK)
                    nc.sync.dma_start(yt_d[:, slp, :, :],
                                      ys_p[:, ja:ja + 2 * BLK, :, :])

            for i in range(NB):
                g, b = divmod(i, BPG)
                if b == 0:
                    tuv = pending.pop(0)
                    ys = spool.tile([H2, G, 2, W], bf16, name="ys", tag="ys")
                    if g + 2 < IMGS // G:
                        pending.append(load_group(g + 2))

                # stage 1: per image 8 MMs / 4 stationary loads; the U/V
                # butterfly happens in PSUM accumulation.
                up = pst.tile([128, 2, BLK, 128], f32, name="up", tag="up")
                vp = pst.tile([128, 2, BLK, 128], f32, name="vp", tag="vp")
                for j in range(BLK):
                    jj = b * BLK + j
                    for p, (pos, neg) in enumerate(((det_s, detn_s),
                                                    (dot_s, dotn_s))):
                        sl_ = tuv[:, jj, p, 0:H2]
                        sr_ = tuv[:, jj, p, H2:224]
                        nc.tensor.matmul(up[0:H2, p, j, 0:H2], sl_,
                                         pos, start=True, stop=False)
                        nc.tensor.matmul(vp[0:H2, p, j, 0:H2], sl_,
                                         pos, start=True, stop=False)
                        nc.tensor.matmul(up[0:H2, p, j, 0:H2], sr_,
                                         pos, start=False, stop=True)
                        nc.tensor.matmul(vp[0:H2, p, j, 0:H2], sr_,
                                         neg, start=False, stop=True)

                # stage 2 + output evac, DEPTH blocks behind (emitted before
                # the U/V evacs so the Ye/Yo release chain has Act/DVE queue
                # priority)
                if len(states) >= DEPTH:
                    stage2_and_out(states.pop(0))

                # evac U/V -> bf16 SBUF (unary PSUM reads: Act + DVE)
                US = upool.tile([H2, 2, BLK, H2], bf16, name="US", tag="US")
                VS = upool.tile([H2, 2, BLK, H2], bf16, name="VS", tag="VS")
                nc.scalar.copy(US[:, :, :, :], up[0:H2, :, :, 0:H2])
                nc.vector.tensor_copy(VS[:, :, :, :], vp[0:H2, :, :, 0:H2])

                states.append((US, VS, ys, b, g))

            for st in states:
                stage2_and_out(st)

    nc.compile()
    return nc


def _prep_inputs(x: np.ndarray):
    """x: [B*C, H, W] fp32 -> per-core input maps."""
    D = _dct2_matrix(H)
    De = D[0::2, 0:H2]  # [112, 112]
    Do = D[1::2, 0:H2]
    de = np.stack([De.T, Do.T], axis=1).astype(bfloat16)      # [112, 2, 112]
    dn = np.stack([-De.T, -Do.T], axis=1).astype(bfloat16)

    A = x[:, 0:H2, :]
    Brev = x[:, 223:111:-1, :]
    u = A + Brev
    v = A - Brev
    # [img, h', 2, 224] with right half columns reversed
    uv = np.empty((x.shape[0], H2, 2, W), dtype=bfloat16)
    uv[:, :, 0, 0:H2] = u[:, :, 0:H2].astype(bfloat16)
    uv[:, :, 0, H2:224] = u[:, :, 223:111:-1].astype(bfloat16)
    uv[:, :, 1, 0:H2] = v[:, :, 0:H2].astype(bfloat16)
    uv[:, :, 1, H2:224] = v[:, :, 223:111:-1].astype(bfloat16)

    in_maps = []
    for i in range(N_CORES):
        # partition-major: [h', img, 2, 224] so each DMA partition row is
        # one contiguous multi-KB run per group
        uvT = np.ascontiguousarray(
            uv[i * IMGS:(i + 1) * IMGS].transpose(1, 0, 2, 3))
        in_maps.append({"uv": uvT, "de": de, "dn": dn})
    return in_maps


def _assemble(results) -> np.ndarray:
    """Per-core yt [112, IMGS, 2, 224] bf16 -> full y [B, C, H, W] fp32."""
    yt = np.concatenate([r["yt"] for r in results], axis=1).astype(np.float32)
    ye = yt[:, :, 0, :]  # [l', N, m]  (m<112: k=2m ; m>=112: k=2(m-112)+1)
    yo = yt[:, :, 1, :]
    y = np.empty((B * C, H, W), np.float32)
    y[:, 0::2, 0::2] = ye[:, :, 0:H2].transpose(1, 2, 0)
    y[:, 1::2, 0::2] = ye[:, :, H2:224].transpose(1, 2, 0)
    y[:, 0::2, 1::2] = yo[:, :, 0:H2].transpose(1, 2, 0)
    y[:, 1::2, 1::2] = yo[:, :, H2:224].transpose(1, 2, 0)
    return y.reshape(B, C, H, W)


def _run(x: np.ndarray, trace: bool = False):
    """x: [B, C, H, W] fp32. Returns (y, BassKernelResults)."""
    if "nc" not in _cache:
        _cache["nc"] = _build()
    nc = _cache["nc"]
    flat = np.ascontiguousarray(x.reshape(B * C, H, W).astype(np.float32))
    in_maps = _prep_inputs(flat)
    res = run_bass_kernel_spmd(nc, in_maps, core_ids=list(range(N_CORES)),
                               trace=trace)
    return _assemble(res.results), res


def kernel(x: np.ndarray) -> np.ndarray:
    y, _ = _run(np.asarray(x))
    return y


# revision 14
# speedup vs baseline: 1.0330x; 1.0330x over previous
"""2D orthonormal DCT-II over [32,64,224,224], data-parallel on 8 TRN2 cores.

Math per image X [224,224]:  Y = D @ X @ D.T  (D = 224-pt orthonormal DCT-II).
Uses the DCT cosine even/odd symmetry D[k, n-1-m] = (-1)^k D[k, m] to fold
both contractions to half size (4x fewer MACs than naive), and runs the
whole pipeline in bf16 (1 cyc/row on the PE at any stream length, half the
HBM traffic of fp32; rel err ~5e-3 vs the 2e-2 gate).

Host prep (layout + first butterfly, 0.2% of the FLOPs):
  u = X[0:112] + X[223:111:-1]   (rows folded)
  v = X[0:112] - X[223:111:-1]
  columns rearranged to [left 112 | right 112 reversed]; arrays stored
  partition-major ([h', img, ...]) so each DMA partition row is one
  contiguous multi-KB run per group.

Device, per 2-image block (all matmuls bf16, fp32 PSUM accum):
  stage 1 (data stationary): per image 8 MMs / 4 weight loads.
  The second butterfly (U = T_left + T_right, V = T_left - T_right) runs on
  the PE itself via PSUM accumulation -- the right-half matmul accumulates
  on top of the left-half one, with a sign-negated DCT stream for V.
  (DVE cannot add two PSUM operands, so folding on the PE is both legal
  and free: same stationary operand feeds the U and V streams.)
    U[w',(ke|ko)] = MM(u_l, DeT) + MM(u_r,  DeT)   (and v_*, DoT for ko)
    V[w',(ke|ko)] = MM(u_l, DeT) + MM(u_r, -DeT)
  evac U,V -> bf16 SBUF (Act / DVE unary copies)
  stage 2 (DCT stationary, 448-col streams over the 2-image block), run
  with a 2-block delay so the U/V evacuation latency is fully hidden:
    Ye[le,(img,k)] = De @ U      Yo[lo,(img,k)] = Do @ V     (2 MMs)
  evac Ye/Yo -> bf16 SBUF -> DRAM (partition-major); host de-interleaves.

PSUM: Up,Vp (1 bank each) + Ye,Yo (1 bank each), all double-buffered = 8
banks, so stage 1 of block i+1 never waits on the evacs of block i.
"""
import numpy as np
from ml_dtypes import bfloat16
import concourse.bacc as bacc
import concourse.mybir as mybir
import concourse.tile as tile
from concourse.bass_utils import run_bass_kernel_spmd

B, C, H, W = 32, 64, 224, 224
N_CORES = 8
IMGS = B * C // N_CORES  # 256 images per core
G = 8                    # images per DMA group
BLK = 2                  # images per PSUM block
NB = IMGS // BLK         # 128 blocks
H2 = 112                 # folded size

f32 = mybir.dt.float32
bf16 = mybir.dt.bfloat16

_cache = {}


def _dct2_matrix(n: int) -> np.ndarray:
    k = np.arange(n)[:, None].astype(np.float64)
    m = np.arange(n)[None, :].astype(np.float64)
    d = np.cos(np.pi * (2.0 * m + 1.0) * k / (2.0 * n))
    scale = np.full((n, 1), np.sqrt(2.0 / n))
    scale[0, 0] = np.sqrt(1.0 / n)
    return (scale * d).astype(np.float32)


def _build():
    nc = bacc.Bacc("TRN2", target_bir_lowering=False, debug=False)
    uv_d = nc.dram_tensor("uv", [H2, IMGS, 2, W], bf16, kind="ExternalInput").ap()
    de_d = nc.dram_tensor("de", [H2, 2, H2], bf16, kind="ExternalInput").ap()
    dn_d = nc.dram_tensor("dn", [H2, 2, H2], bf16, kind="ExternalInput").ap()
    yt_d = nc.dram_tensor("yt", [H2, IMGS, 2, W], bf16, kind="ExternalOutput").ap()

    with tile.TileContext(nc) as tc:
        with (
            tc.tile_pool(name="consts", bufs=1) as cpool,
            tc.tile_pool(name="xin", bufs=4) as xpool,
            tc.tile_pool(name="uv", bufs=3) as upool,
            tc.tile_pool(name="yout", bufs=2) as spool,
            tc.tile_pool(name="pst", bufs=2, space="PSUM") as pst,
            tc.tile_pool(name="psy", bufs=2, space="PSUM") as psy,
        ):
            # constants: DeT/DoT and negations [112, 2, 112]
            dct_s = cpool.tile([H2, 2, H2], bf16)
            dctn_s = cpool.tile([H2, 2, H2], bf16)
            nc.gpsimd.dma_start(dct_s, de_d)
            nc.gpsimd.dma_start(dctn_s, dn_d)
            det_s, dot_s = dct_s[:, 0, :], dct_s[:, 1, :]
            detn_s, dotn_s = dctn_s[:, 0, :], dctn_s[:, 1, :]

            # PE warmup: ~4us of junk matmuls to ramp the p-state before
            # the real work starts.
            junk_w = cpool.tile([H2, 128], bf16)
            junk_m = cpool.tile([H2, 448], bf16)
            nc.vector.memset(junk_w, 0)
            nc.vector.memset(junk_m, 0)
            for r in range(20):
                wp = psy.tile([128, 2, BLK, 256], f32, name=f"warm{r}",
                              tag="y2")
                nc.tensor.matmul(wp[:, r % 2, :, 0:W], junk_w, junk_m,
                                 start=True, stop=True)

            def load_group(g):
                t = xpool.tile([H2, G, 2, W], bf16, name="tuv", tag="tuv")
                sl = slice(g * G, (g + 1) * G)
                nc.gpsimd.dma_start(t, uv_d[:, sl, :, :])
                return t

            BPG = G // BLK   # blocks per group
            DEPTH = 2        # s2 trails stage 1 by this many blocks
            states = []      # (US, VS, ys, b, g)
            pending = [load_group(0), load_group(1), load_group(2)]
            tuv = None
            ys = None

            def stage2_and_out(st):
                USp, VSp, ys_p, b_p, g_p = st
                y2 = psy.tile([128, 2, BLK, 256], f32, name="y2", tag="y2")
                nc.tensor.matmul(y2[0:H2, 0, :, 0:W], det_s,
                                 USp[:, :, :, :].transpose([0, 2, 1, 3]),
                                 start=True, stop=True)
                nc.tensor.matmul(y2[0:H2, 1, :, 0:W], dot_s,
                                 VSp[:, :, :, :].transpose([0, 2, 1, 3]),
                                 start=True, stop=True)
                j0 = b_p * BLK
                # one merged output evac (DVE): [l', par, img, k] -> ys
                nc.vector.tensor_copy(
                    ys_p[:, j0:j0 + BLK, :, :].transpose([0, 2, 1, 3]),
                    y2[0:H2, :, :, 0:W])
                if b_p % 2 == 1:
                    # flush a 4-image output chunk (keeps the out stream
                    # smooth and the final drain short)
                    ja = (b_p - 1) * BLK
                    slp = slice(g_p * G + ja, g_p * G + ja + 2 * BLK)
                    nc.sync.dma_start(yt_d[:, slp, :, :],
                                      ys_p[:, ja:ja + 2 * BLK, :, :])

            for i in range(NB):
                g, b = divmod(i, BPG)
                if b == 0:
                    tuv = pending.pop(0)
                    ys = spool.tile([H2, G, 2, W], bf16, name="ys", tag="ys")
                    if g + 3 < IMGS // G:
                        pending.append(load_group(g + 3))

                # stage 1: per image 8 MMs / 4 stationary loads; the U/V
                # butterfly happens in PSUM accumulation.
                up = pst.tile([128, 2, BLK, 128], f32, name="up", tag="up")
                vp = pst.tile([128, 2, BLK, 128], f32, name="vp", tag="vp")
                for j in range(BLK):
                    jj = b * BLK + j
                    for p, (pos, neg) in enumerate(((det_s, detn_s),
                                                    (dot_s, dotn_s))):
                        sl_ = tuv[:, jj, p, 0:H2]
                        sr_ = tuv[:, jj, p, H2:224]
                        nc.tensor.matmul(up[0:H2, p, j, 0:H2], sl_,
                                         pos, start=True, stop=False)
                        nc.tensor.matmul(vp[0:H2, p, j, 0:H2], sl_,
                                         pos, start=True, stop=False)
                        nc.tensor.matmul(up[0:H2, p, j, 0:H2], sr_,
                                         pos, start=False, stop=True)
                        nc.tensor.matmul(vp[0:H2, p, j, 0:H2], sr_,
                                         neg, start=False, stop=True)

                # stage 2 + output evac, DEPTH blocks behind (emitted before
                # the U/V evacs so the Ye/Yo release chain has Act/DVE queue
                # priority)
                if len(states) >= DEPTH:
                    stage2_and_out(states.pop(0))

                # evac U/V -> bf16 SBUF (unary PSUM reads: Act + DVE)
                US = upool.tile([H2, 2, BLK, H2], bf16, name="US", tag="US")
                VS = upool.tile([H2, 2, BLK, H2], bf16, name="VS", tag="VS")
                nc.scalar.copy(US[:, :, :, :], up[0:H2, :, :, 0:H2])
                nc.scalar.copy(VS[:, :, :, :], vp[0:H2, :, :, 0:H2])

                states.append((US, VS, ys, b, g))

            for st in states:
                stage2_and_out(st)

    nc.compile()
    return nc


def _prep_inputs(x: np.ndarray):
    """x: [B*C, H, W] fp32 -> per-core input maps."""
    D = _dct2_matrix(H)
    De = D[0::2, 0:H2]  # [112, 112]
    Do = D[1::2, 0:H2]
    de = np.stack([De.T, Do.T], axis=1).astype(bfloat16)      # [112, 2, 112]
    dn = np.stack([-De.T, -Do.T], axis=1).astype(bfloat16)

    A = x[:, 0:H2, :]
    Brev = x[:, 223:111:-1, :]
    u = A + Brev
    v = A - Brev
    # [img, h', 2, 224] with right half columns reversed
    uv = np.empty((x.shape[0], H2, 2, W), dtype=bfloat16)
    uv[:, :, 0, 0:H2] = u[:, :, 0:H2].astype(bfloat16)
    uv[:, :, 0, H2:224] = u[:, :, 223:111:-1].astype(bfloat16)
    uv[:, :, 1, 0:H2] = v[:, :, 0:H2].astype(bfloat16)
    uv[:, :, 1, H2:224] = v[:, :, 223:111:-1].astype(bfloat16)

    in_maps = []
    for i in range(N_CORES):
        # partition-major: [h', img, 2, 224] so each DMA partition row is
        # one contiguous multi-KB run per group
        uvT = np.ascontiguousarray(
            uv[i * IMGS:(i + 1) * IMGS].transpose(1, 0, 2, 3))
        in_maps.append({"uv": uvT, "de": de, "dn": dn})
    return in_maps


def _assemble(results) -> np.ndarray:
    """Per-core yt [112, IMGS, 2, 224] bf16 -> full y [B, C, H, W] fp32."""
    yt = np.concatenate([r["yt"] for r in results], axis=1).astype(np.float32)
    ye = yt[:, :, 0, :]  # [l', N, m]  (m<112: k=2m ; m>=112: k=2(m-112)+1)
    yo = yt[:, :, 1, :]
    y = np.empty((B * C, H, W), np.float32)
    y[:, 0::2, 0::2] = ye[:, :, 0:H2].transpose(1, 2, 0)
    y[:, 1::2, 0::2] = ye[:, :, H2:224].transpose(1, 2, 0)
    y[:, 0::2, 1::2] = yo[:, :, 0:H2].transpose(1, 2, 0)
    y[:, 1::2, 1::2] = yo[:, :, H2:224].transpose(1, 2, 0)
    return y.reshape(B, C, H, W)


def _run(x: np.ndarray, trace: bool = False):
    """x: [B, C, H, W] fp32. Returns (y, BassKernelResults)."""
    if "nc" not in _cache:
        _cache["nc"] = _build()
    nc = _cache["nc"]
    flat = np.ascontiguousarray(x.reshape(B * C, H, W).astype(np.float32))
    in_maps = _prep_inputs(flat)
    res = run_bass_kernel_spmd(nc, in_maps, core_ids=list(range(N_CORES)),
                               trace=trace)
    return _assemble(res.results), res


def kernel(x: np.ndarray) -> np.ndarray:
    y, _ = _run(np.asarray(x))
    return y
